# revision 17
# baseline (speedup 1.0000x reference)
"""Trainium2 Bass kernel for the EquivarLayer GNN message-passing problem.

Computation (see reference):
    ix      = (p3[j] + d3[:, :, None]) * i1[:, None, :]        # (n_pairs, 3, C)
    p3_agg  = segment_sum(ix, i, n_atoms)                      # (n_atoms, 3, C)
    p3_new  = einsum('axc,dc->axd', p3_agg, W)
    dotted  = einsum('ixr,ixr->ir', p3_new, p3_new)

Strategy (8 NeuronCores, SPMD single program):
  * Receiver-atom sharding: core r owns atoms [r*6250, (r+1)*6250). Host sorts
    edges by receiver i and buckets them into 49 blocks of 128 atoms per core.
    Each block's edge list is padded to a uniform K tiles of 128 edges so all
    cores run the identical program. No collectives needed - per-core
    aggregates are disjoint.
  * Per block: dma_gather pulls p3[j] rows (bf16, padded to 512B for full DMA
    descriptor efficiency) with edges-on-partitions (slot s -> partition
    s%128, tile s//128). dma_gather indices are int16, so the p3 table is
    split at row 32768: each block's slots are ordered [low-j tiles | high-j
    tiles] and two gathers run per block.
  * A fused scalar_tensor_tensor computes (g + d3) * i1 per (tile, x); a
    one-hot (iota == local_id) matmul scatters/accumulates into PSUM (the
    segment sum runs on the TensorEngine, resolving duplicate receivers).
  * Epilogue per block: PE transpose -> f32 FF matmul with W^T -> square/add
    for the dot -> transpose back -> DMA out.
"""

import os

import numpy as np

import concourse.bass as bass
import concourse.tile as tile
from concourse import bacc, mybir
from concourse.bass_utils import run_bass_kernel_spmd
from concourse.masks import make_identity

BF16 = mybir.dt.np(mybir.dt.bfloat16)

N_ATOMS = 50000
N_PAIRS = 800000
C = 64
NCORES = 8
APC = N_ATOMS // NCORES          # 6250 atoms per core
BLK = 128                        # atoms per block
NBLK = (APC + BLK - 1) // BLK    # 49 blocks per core (last block: 106 atoms)
ROW = 256                        # padded gather row elements (bf16) = 512B
XC = 3 * C                       # 192
HALF = 32768                     # int16 index limit for dma_gather

_prog_cache: dict = {}


def build_program(K_lo: int, K_hi: int, repeat: int = 1):
    f32 = mybir.dt.float32
    bf16 = mybir.dt.bfloat16
    i32 = mybir.dt.int32
    i16 = mybir.dt.int16
    ADD = mybir.AluOpType.add
    MULT = mybir.AluOpType.mult
    ISEQ = mybir.AluOpType.is_equal

    K = K_lo + K_hi
    S_lo, S_hi = K_lo * 128, K_hi * 128
    NB_HI = N_ATOMS - HALF
    # per-partition u16 lanes in the packed block buffer:
    #   [Wl idxlo(i16) | Wh idxhi(i16) | 2K lid(f32) | 6K d3(f32) | 64K i1(bf16)]
    Wl, Wh = S_lo // 16, S_hi // 16
    OFF_LID = Wl + Wh
    OFF_D3 = OFF_LID + 2 * K
    OFF_I1 = OFF_D3 + 6 * K
    BW = OFF_I1 + C * K

    nc = bacc.Bacc()
    blk_d = nc.declare_dram_parameter("blk", [NBLK, 128, BW], bf16, isOutput=False)
    p3a_d = nc.declare_dram_parameter("p3a", [HALF, ROW], bf16, isOutput=False)
    p3b_d = nc.declare_dram_parameter("p3b", [NB_HI, ROW], bf16, isOutput=False)
    wt_d = nc.declare_dram_parameter("wt", [C, C], f32, isOutput=False)
    p3n_d = nc.declare_dram_parameter("p3n", [APC, XC], f32, isOutput=True)
    dot_d = nc.declare_dram_parameter("dot", [APC, C], f32, isOutput=True)

    with tile.TileContext(nc) as tc:
        with (
            tc.tile_pool(name="const", bufs=1) as constp,
            tc.tile_pool(name="io", bufs=3) as iop,
            tc.tile_pool(name="msg", bufs=2) as msgp,
            tc.tile_pool(name="oh", bufs=4) as ohp,
            tc.tile_pool(name="ep", bufs=2) as epp,
            tc.tile_pool(name="psagg", bufs=2, space="PSUM") as psagg,
            tc.tile_pool(name="pstmp", bufs=4, space="PSUM") as pstmp,
        ):
            ident = constp.tile([128, 128], f32)
            make_identity(nc, ident[:])
            iota_i = constp.tile([128, 128], i32)
            nc.gpsimd.iota(iota_i[:], pattern=[[1, 128]], base=0, channel_multiplier=0)
            iota_b = constp.tile([128, 128], bf16)
            nc.vector.tensor_copy(iota_b[:], iota_i[:])
            wts = constp.tile([C, C], f32)
            nc.sync.dma_start(out=wts[:], in_=wt_d[:, :])

            for b in [bb for _ in range(repeat) for bb in range(NBLK)]:
                rows = min(BLK, APC - b * BLK)

                blk_t = iop.tile([128, BW], bf16, tag="blk")
                nc.sync.dma_start(out=blk_t[:], in_=blk_d[b])

                idxlo_v = blk_t[:, 0:Wl].bitcast(i16)
                idxhi_v = blk_t[:, Wl:OFF_LID].bitcast(i16)
                lid_v = blk_t[:, OFF_LID : OFF_LID + 2 * K].bitcast(f32)
                d3_v = blk_t[:, OFF_D3 : OFF_D3 + 6 * K].bitcast(f32)
                i1_t = blk_t[:, OFF_I1:BW]

                msg = msgp.tile([128, K * ROW], bf16, tag="msg")
                msg3 = msg[:, :].rearrange("p (t e) -> p t e", e=ROW)
                # one dma_gather handles at most 1024 indices (SWDGE ring cap)
                for base, n_t, tab, idxv in (
                    (0, K_lo, p3a_d, idxlo_v),
                    (K_lo, K_hi, p3b_d, idxhi_v),
                ):
                    done = 0
                    while done < n_t:
                        ch = min(8, n_t - done)
                        nidx = ch * 128
                        nc.gpsimd.dma_gather(
                            out_ap=msg3[:, base + done : base + done + ch, :],
                            in_ap=tab[:, :],
                            idxs_ap=idxv[:, done * 8 : done * 8 + ch * 8],
                            num_idxs=nidx,
                            num_idxs_reg=nidx,
                            elem_size=ROW,
                        )
                        done += ch

                agg = psagg.tile([128, XC], f32, tag="agg")
                for t in range(K):
                    for x in range(3):
                        sl = msg[:, t * ROW + x * C : t * ROW + (x + 1) * C]
                        nc.vector.scalar_tensor_tensor(
                            out=sl,
                            in0=sl,
                            scalar=d3_v[:, t * 3 + x : t * 3 + x + 1],
                            in1=i1_t[:, t * C : (t + 1) * C],
                            op0=ADD,
                            op1=MULT,
                        )
                    oh = ohp.tile([128, 128], bf16, tag="oh")
                    nc.any.tensor_scalar(
                        out=oh[:],
                        in0=iota_b[:],
                        scalar1=lid_v[:, t : t + 1],
                        scalar2=None,
                        op0=ISEQ,
                    )
                    nc.tensor.matmul(
                        out=agg[:],
                        lhsT=oh[:],
                        rhs=msg[:, t * ROW : t * ROW + XC],
                        start=(t == 0),
                        stop=(t == K - 1),
                    )

                # ---- epilogue: FF (x @ W.T) + dot, then store ----
                aggS = epp.tile([128, XC], f32, tag="aggS")
                nc.any.tensor_copy(aggS[:], agg[:])
                # transpose all 3 x-slices into ONE psum tile, copy once
                trall = pstmp.tile([C, 3 * 128], f32, tag="tmp")
                for x in range(3):
                    nc.tensor.transpose(
                        out=trall[:, x * 128 : (x + 1) * 128],
                        in_=aggS[:, x * C : (x + 1) * C],
                        identity=ident[:],
                    )
                trS = epp.tile([C, 3 * 128], f32, tag="trS")
                nc.any.tensor_copy(trS[:], trall[:])
                ffall = pstmp.tile([C, 3 * 128], f32, tag="tmp")
                for x in range(3):
                    nc.tensor.matmul(
                        out=ffall[:, x * 128 : (x + 1) * 128],
                        lhsT=wts[:],
                        rhs=trS[:, x * 128 : (x + 1) * 128],
                        start=True,
                        stop=True,
                    )
                ffS = epp.tile([C, 3 * 128], f32, tag="ffS")
                nc.any.tensor_copy(ffS[:], ffall[:])
                sq = epp.tile([C, 3 * 128], f32, tag="sq")
                nc.scalar.square(sq[:], ffS[:])
                dS = epp.tile([C, 128], f32, tag="dS")
                nc.vector.tensor_tensor(
                    out=dS[:], in0=sq[:, 0:128], in1=sq[:, 128:256], op=ADD
                )
                nc.vector.tensor_tensor(
                    out=dS[:], in0=dS[:], in1=sq[:, 256:384], op=ADD
                )
                # transpose back: p3_new cols + dotted into ONE psum tile
                ptr = pstmp.tile([128, 4 * C], f32, tag="tmp")
                for x in range(3):
                    nc.tensor.transpose(
                        out=ptr[:, x * C : (x + 1) * C],
                        in_=ffS[:, x * 128 : (x + 1) * 128],
                        identity=ident[:C, :C],
                    )
                nc.tensor.transpose(
                    out=ptr[:, 3 * C : 4 * C], in_=dS[:], identity=ident[:C, :C]
                )
                outA = epp.tile([128, 4 * C], f32, tag="outA")
                nc.any.tensor_copy(outA[:], ptr[:])
                nc.sync.dma_start(
                    out=p3n_d[b * BLK : b * BLK + rows, :], in_=outA[:rows, 0:XC]
                )
                nc.scalar.dma_start(
                    out=dot_d[b * BLK : b * BLK + rows, :], in_=outA[:rows, XC : 4 * C]
                )
    return nc


def get_program(K_lo: int, K_hi: int):
    key = (K_lo, K_hi)
    if key not in _prog_cache:
        nc = build_program(K_lo, K_hi)
        nc.finalize()
        _prog_cache[key] = nc
    return _prog_cache[key]


def _wrap16(idx_i16: np.ndarray) -> np.ndarray:
    """(..., S) int16 -> (..., 128, S//16): index s at partition s%16, col s//16,
    replicated across the 8 groups of 16 partitions."""
    *lead, S = idx_i16.shape
    w = idx_i16.reshape(*lead, S // 16, 16)
    w = np.moveaxis(w, -1, -2)  # (..., 16, S//16)
    return np.ascontiguousarray(
        np.broadcast_to(w[..., None, :, :], (*lead, 8, 16, S // 16)).reshape(
            *lead, 128, S // 16
        )
    )


def prepare_inputs(ind_2, p3, i1, d3, W):
    """Host-side shard/sort/pad. Returns (K_lo, K_hi, per-core in_maps)."""
    ind_2 = np.asarray(ind_2)
    i = ind_2[:, 0].astype(np.int64)
    j = ind_2[:, 1].astype(np.int64)
    p3 = np.asarray(p3, dtype=np.float32)
    i1 = np.asarray(i1, dtype=np.float32)
    d3 = np.asarray(d3, dtype=np.float32)
    W = np.asarray(W, dtype=np.float32)

    core = i // APC
    rem = i - core * APC
    blk = rem // BLK
    lid_all = (rem - blk * BLK).astype(np.float32)
    bucket = core * NBLK + blk
    grp = bucket * 2 + (j >= HALF)
    order = np.argsort(grp, kind="stable")
    grp_s = grp[order]
    j_s = j[order]
    lid_s = lid_all[order]

    counts2 = np.bincount(grp_s, minlength=NCORES * NBLK * 2)
    n_lo = counts2[0::2]
    n_hi = counts2[1::2]
    K_lo = max(1, int(np.ceil(n_lo.max() / 128)))
    K_hi = max(1, int(np.ceil(n_hi.max() / 128)))
    K = K_lo + K_hi
    S_lo, S_hi = K_lo * 128, K_hi * 128
    S = S_lo + S_hi

    starts2 = np.zeros(NCORES * NBLK * 2, np.int64)
    np.cumsum(counts2[:-1], out=starts2[1:])
    pos = np.arange(j_s.size, dtype=np.int64) - starts2[grp_s]
    is_hi_s = (grp_s % 2).astype(np.int64)
    slot_in_bucket = pos + is_hi_s * S_lo
    slot = (grp_s // 2) * S + slot_in_bucket
    NS = NCORES * NBLK * S

    idxv = np.zeros(NS, np.int16)
    idxv[slot] = (j_s - is_hi_s * HALF).astype(np.int16)
    lidf = np.full(NS, 255.0, np.float32)
    lidf[slot] = lid_s
    d3s = np.zeros((NS, 3), np.float32)
    d3s[slot] = d3[order]
    i1s = np.zeros((NS, C), BF16)
    i1s[slot] = i1[order].astype(BF16)

    # slot s within a bucket maps to (partition s % 128, tile s // 128)
    def to_tiles(arr, inner):
        # (NC*NBLK*S, inner) -> (NC, NBLK, 128, K*inner)
        a = arr.reshape(NCORES, NBLK, K, 128, inner)
        a = np.swapaxes(a, 2, 3)  # (NC, NBLK, 128, K, inner)
        return np.ascontiguousarray(a).reshape(NCORES, NBLK, 128, K * inner)

    lid_t = to_tiles(lidf.reshape(NS, 1), 1)
    d3_t = to_tiles(d3s, 3)
    i1_t = to_tiles(i1s, C)

    idxv2 = idxv.reshape(NCORES, NBLK, S)
    idxlo_w = _wrap16(idxv2[:, :, :S_lo])  # (NC, NBLK, 128, S_lo//16)
    idxhi_w = _wrap16(idxv2[:, :, S_lo:])

    Wl, Wh = S_lo // 16, S_hi // 16
    BW = Wl + Wh + 2 * K + 6 * K + C * K
    blkbuf = np.empty((NCORES, NBLK, 128, BW), np.uint16)
    blkbuf[..., 0:Wl] = idxlo_w.view(np.uint16)
    blkbuf[..., Wl : Wl + Wh] = idxhi_w.view(np.uint16)
    o = Wl + Wh
    blkbuf[..., o : o + 2 * K] = lid_t.view(np.uint16)
    blkbuf[..., o + 2 * K : o + 8 * K] = d3_t.view(np.uint16)
    blkbuf[..., o + 8 * K : BW] = i1_t.view(np.uint16)

    p3r = p3.reshape(N_ATOMS, XC).astype(BF16)
    p3a = np.zeros((HALF, ROW), BF16)
    p3a[:, :XC] = p3r[:HALF]
    p3b = np.zeros((N_ATOMS - HALF, ROW), BF16)
    p3b[:, :XC] = p3r[HALF:]
    wt = np.ascontiguousarray(W.T.astype(np.float32))  # lhsT[c, d] = W[d, c]

    in_maps = [
        {"blk": blkbuf[r].view(BF16), "p3a": p3a, "p3b": p3b, "wt": wt}
        for r in range(NCORES)
    ]
    return K_lo, K_hi, in_maps


def assemble_outputs(results):
    p3n = np.concatenate([results[r]["p3n"] for r in range(NCORES)], axis=0)
    dot = np.concatenate([results[r]["dot"] for r in range(NCORES)], axis=0)
    return (
        np.ascontiguousarray(p3n.reshape(N_ATOMS, 3, C).astype(np.float32)),
        np.ascontiguousarray(dot.astype(np.float32)),
    )


def kernel(ind_2, p3, i1, d3, W):
    K_lo, K_hi, in_maps = prepare_inputs(ind_2, p3, i1, d3, W)
    nc = get_program(K_lo, K_hi)
    trace = bool(int(os.environ.get("KERNEL_TRACE", "0")))
    res = run_bass_kernel_spmd(nc, in_maps, list(range(NCORES)), trace=trace)
    if trace:
        kernel.last_exec_time_ns = res.exec_time_ns
        kernel.last_results = res
        print(f"[kernel] exec_time_ns={res.exec_time_ns} mean={res.mean_exec_time_ns}")
    return assemble_outputs(res.results)


kernel.last_exec_time_ns = None
kernel.last_results = None
